# revision 3
# baseline (speedup 1.0000x reference)
"""Trainium2 Bass kernel for nn_Decoder: out = (x - b_pre) @ W^T.

Shapes (hardcoded): x [8192, 32768] f32, W [768, 32768] f32, b_pre [32768] f32
-> out [8192, 768] f32.

Strategy: data-parallel over the 8192 token rows across 8 NeuronCores
(1024 rows each), W replicated. The TensorE contracts over the partition
axis, so both operands are fed with the contraction dim (d = 32768) on
partitions: the host pre-transposes each x shard to xT [32768, 1024] and
W to wT [32768, 768] (cheap, ~0.2 s/shard). b_pre is folded into x on
the host (x - b_pre) before the transpose; with the reference's
b_pre == 0 this is bitwise a no-op.

Per core the kernel streams d in 256 chunks of 128, accumulating the
full contraction in PSUM: two n-blocks of 512 token rows, each holding
4 psum tiles of [128, 768] (2 banks apiece = all 8 banks). Every d-chunk
issues 8 matmuls (4 stationary xT slices x 2 moving wT slices of
512/256 cols). PSUM is evicted once per output tile through SBUF.
"""

import os
import sys

if "/opt/trn_rl_repo" not in sys.path:
    sys.path.insert(0, "/opt/trn_rl_repo")

import numpy as np

N_TOK = 8192
D_IN = 32768
D_OUT = 768
N_CORES = 8
N_SHARD = N_TOK // N_CORES          # 1024 token rows per core
P = 128
D_CHUNKS = D_IN // P                # 256
N_SUPER = 512                       # token rows resident in PSUM at once
N_SUPERS = N_SHARD // N_SUPER       # 2
N_CH = N_SUPER // P                 # 4 psum tiles per n-block

# Matmul input dtype knob: "float32" (exact, 4 cyc/row) or "float32r"
# (TF32-like, 1 cyc/row at N>=256).
MM_DTYPE = os.environ.get("KERNEL_MM_DTYPE", "float32")
# "psum": full-K accumulation in PSUM, W streamed twice (simplest).
# "sbuf": d-super blocking, output accumulated in SBUF, min DMA traffic.
DESIGN = os.environ.get("KERNEL_DESIGN", "psum")

LAST_RESULTS = None  # BassKernelResults of the most recent kernel() call


def _build_bass_sbuf():
    """Design 1: stream xT and wT exactly once in d-supers of 1024 rows;
    accumulate the [1024, 768] output in SBUF across d-supers (DVE adds
    PSUM into the resident C tiles)."""
    import concourse.mybir as mybir
    import concourse.tile as tile
    from concourse import bacc

    dt_mm = getattr(mybir.dt, MM_DTYPE)
    f32 = mybir.dt.float32
    DSUP = 8                       # d-chunks per super
    NSUP = D_CHUNKS // DSUP        # 32 supers
    NCH = N_SHARD // P             # 8 output row-chunks

    nc = bacc.Bacc(None, target_bir_lowering=False)
    xT = nc.dram_tensor("xT", [D_IN, N_SHARD], dt_mm, kind="ExternalInput")
    wT = nc.dram_tensor("wT", [D_IN, D_OUT], dt_mm, kind="ExternalInput")
    out = nc.dram_tensor("out", [N_SHARD, D_OUT], f32, kind="ExternalOutput")

    with tile.TileContext(nc) as tc:
        with (
            tc.tile_pool(name="xs", bufs=2) as xpool,
            tc.tile_pool(name="ws", bufs=2) as wpool,
            tc.tile_pool(name="c", bufs=1) as cpool,
            tc.tile_pool(name="psum", bufs=3, space="PSUM") as ppool,
        ):
            cts = [
                cpool.tile([P, D_OUT], f32, name=f"c{i}") for i in range(NCH)
            ]
            for ds in range(NSUP):
                xs = xpool.tile([P, DSUP, N_SHARD], dt_mm, name="xs")
                ws = wpool.tile([P, DSUP, D_OUT], dt_mm, name="ws")
                for j in range(DSUP):
                    row = (ds * DSUP + j) * P
                    nc.sync.dma_start(xs[:, j, :], xT[row:row + P, :])
                    nc.sync.dma_start(ws[:, j, :], wT[row:row + P, :])
                for nch in range(NCH):
                    ps = ppool.tile([P, D_OUT], f32, name="ps")
                    for j in range(DSUP):
                        lhsT = xs[:, j, nch * P:(nch + 1) * P]
                        nc.tensor.matmul(
                            ps[:, 0:512], lhsT, ws[:, j, 0:512],
                            start=(j == 0), stop=(j == DSUP - 1),
                        )
                        nc.tensor.matmul(
                            ps[:, 512:D_OUT], lhsT, ws[:, j, 512:D_OUT],
                            start=(j == 0), stop=(j == DSUP - 1),
                        )
                    if ds == 0:
                        nc.vector.tensor_copy(cts[nch][:], ps[:])
                    else:
                        nc.vector.tensor_add(cts[nch][:], cts[nch][:], ps[:])
            for nch in range(NCH):
                nc.sync.dma_start(out[nch * P:(nch + 1) * P, :], cts[nch][:])

    nc.compile()
    return nc


def _build_bass():
    if DESIGN == "sbuf":
        return _build_bass_sbuf()
    import concourse.mybir as mybir
    import concourse.tile as tile
    from concourse import bacc

    dt_mm = getattr(mybir.dt, MM_DTYPE)
    f32 = mybir.dt.float32

    nc = bacc.Bacc(None, target_bir_lowering=False)
    xT = nc.dram_tensor("xT", [D_IN, N_SHARD], dt_mm, kind="ExternalInput")
    wT = nc.dram_tensor("wT", [D_IN, D_OUT], dt_mm, kind="ExternalInput")
    out = nc.dram_tensor("out", [N_SHARD, D_OUT], f32, kind="ExternalOutput")

    with tile.TileContext(nc) as tc:
        with (
            tc.tile_pool(name="xt", bufs=4) as xpool,
            tc.tile_pool(name="wt", bufs=4) as wpool,
            tc.tile_pool(name="ot", bufs=4) as opool,
            tc.tile_pool(name="psum", bufs=1, space="PSUM") as ppool,
        ):
            for ns in range(N_SUPERS):
                psums = [
                    ppool.tile([P, D_OUT], f32, name=f"psum{i}")
                    for i in range(N_CH)
                ]
                for dc in range(D_CHUNKS):
                    xt = xpool.tile([P, N_SUPER], dt_mm)
                    wt = wpool.tile([P, D_OUT], dt_mm)
                    nc.sync.dma_start(
                        xt[:],
                        xT[dc * P:(dc + 1) * P, ns * N_SUPER:(ns + 1) * N_SUPER],
                    )
                    nc.sync.dma_start(wt[:], wT[dc * P:(dc + 1) * P, :])
                    st = dc == 0
                    sp = dc == D_CHUNKS - 1
                    for nch in range(N_CH):
                        lhsT = xt[:, nch * P:(nch + 1) * P]
                        nc.tensor.matmul(
                            psums[nch][:, 0:512], lhsT, wt[:, 0:512],
                            start=st, stop=sp,
                        )
                        nc.tensor.matmul(
                            psums[nch][:, 512:D_OUT], lhsT, wt[:, 512:D_OUT],
                            start=st, stop=sp,
                        )
                for nch in range(N_CH):
                    ot = opool.tile([P, D_OUT], f32)
                    nc.vector.tensor_copy(ot[:], psums[nch][:])
                    base = ns * N_SUPER + nch * P
                    nc.sync.dma_start(out[base:base + P, :], ot[:])

    nc.compile()
    return nc


def kernel(x: np.ndarray, W: np.ndarray, b_pre: np.ndarray) -> np.ndarray:
    global LAST_RESULTS
    from concourse.bass_utils import run_bass_kernel_spmd

    x = np.asarray(x, dtype=np.float32)
    W = np.asarray(W, dtype=np.float32)
    b_pre = np.asarray(b_pre, dtype=np.float32)

    # Fold the pre-bias on the host (exact no-op for b_pre == 0).
    if b_pre.any():
        x = x - b_pre[None, :]

    wTc = np.ascontiguousarray(W.T)  # [D_IN, D_OUT]
    in_maps = []
    for c in range(N_CORES):
        shard = x[c * N_SHARD:(c + 1) * N_SHARD]
        in_maps.append({
            "xT": np.ascontiguousarray(shard.T),  # [D_IN, N_SHARD]
            "wT": wTc,
        })

    nc = _build_bass()
    LAST_RESULTS = run_bass_kernel_spmd(
        nc, in_maps, core_ids=list(range(N_CORES)),
        tmpdir=os.environ.get("KERNEL_TRACE_DIR") or None,
    )
    return np.concatenate(
        [LAST_RESULTS.results[c]["out"] for c in range(N_CORES)], axis=0
    )


# revision 5
# speedup vs baseline: 3.4612x; 3.4612x over previous
"""Trainium2 Bass kernel for nn_Decoder: out = (x - b_pre) @ W^T.

Shapes (hardcoded): x [8192, 32768] f32, W [768, 32768] f32, b_pre [32768] f32
-> out [8192, 768] f32.

Strategy: data-parallel over the 8192 token rows across 8 NeuronCores
(1024 rows each), W replicated. The TensorE contracts over the partition
axis, so both operands are fed with the contraction dim (d = 32768) on
partitions: the host pre-transposes each x shard to xT [32768, 1024] and
W to wT [32768, 768] (cheap, ~0.2 s/shard). b_pre is folded into x on
the host (x - b_pre) before the transpose; with the reference's
b_pre == 0 this is bitwise a no-op.

Per core the kernel streams d in 256 chunks of 128, accumulating the
full contraction in PSUM: two n-blocks of 512 token rows, each holding
4 psum tiles of [128, 768] (2 banks apiece = all 8 banks). Every d-chunk
issues 8 matmuls (4 stationary xT slices x 2 moving wT slices of
512/256 cols). PSUM is evicted once per output tile through SBUF.
"""

import os
import sys

if "/opt/trn_rl_repo" not in sys.path:
    sys.path.insert(0, "/opt/trn_rl_repo")

import numpy as np

N_TOK = 8192
D_IN = 32768
D_OUT = 768
N_CORES = 8
N_SHARD = N_TOK // N_CORES          # 1024 token rows per core
P = 128
D_CHUNKS = D_IN // P                # 256
N_SUPER = 512                       # token rows resident in PSUM at once
N_SUPERS = N_SHARD // N_SUPER       # 2
N_CH = N_SUPER // P                 # 4 psum tiles per n-block

# Matmul input dtype knob: "float32" (exact, 4 cyc/row) or "float32r"
# (TF32-like, 1 cyc/row at N>=256).
MM_DTYPE = os.environ.get("KERNEL_MM_DTYPE", "float32")
# "psum": full-K accumulation in PSUM, W streamed twice (simplest).
# "sbuf": d-super blocking, output accumulated in SBUF, min DMA traffic.
DESIGN = os.environ.get("KERNEL_DESIGN", "psum")

LAST_RESULTS = None  # BassKernelResults of the most recent kernel() call


def _build_bass_sbuf():
    """Design 1: stream xT and wT exactly once in d-supers of 1024 rows;
    accumulate the [1024, 768] output in SBUF across d-supers (DVE adds
    PSUM into the resident C tiles)."""
    import concourse.mybir as mybir
    import concourse.tile as tile
    from concourse import bacc

    dt_mm = getattr(mybir.dt, MM_DTYPE)
    f32 = mybir.dt.float32
    DSUP = 8                       # d-chunks per super
    NSUP = D_CHUNKS // DSUP        # 32 supers
    NCH = N_SHARD // P             # 8 output row-chunks

    nc = bacc.Bacc(None, target_bir_lowering=False)
    xT = nc.dram_tensor("xT", [D_IN, N_SHARD], dt_mm, kind="ExternalInput")
    wT = nc.dram_tensor("wT", [D_IN, D_OUT], dt_mm, kind="ExternalInput")
    out = nc.dram_tensor("out", [N_SHARD, D_OUT], f32, kind="ExternalOutput")

    with tile.TileContext(nc) as tc:
        with (
            tc.tile_pool(name="xs", bufs=2) as xpool,
            tc.tile_pool(name="ws", bufs=2) as wpool,
            tc.tile_pool(name="c", bufs=1) as cpool,
            tc.tile_pool(name="psum", bufs=3, space="PSUM") as ppool,
        ):
            cts = [
                cpool.tile([P, D_OUT], f32, name=f"c{i}") for i in range(NCH)
            ]
            for ds in range(NSUP):
                xs = xpool.tile([P, DSUP, N_SHARD], dt_mm, name="xs")
                ws = wpool.tile([P, DSUP, D_OUT], dt_mm, name="ws")
                for j in range(DSUP):
                    row = (ds * DSUP + j) * P
                    nc.sync.dma_start(xs[:, j, :], xT[row:row + P, :])
                    nc.sync.dma_start(ws[:, j, :], wT[row:row + P, :])
                for nch in range(NCH):
                    ps = ppool.tile([P, D_OUT], f32, name="ps")
                    for j in range(DSUP):
                        lhsT = xs[:, j, nch * P:(nch + 1) * P]
                        nc.tensor.matmul(
                            ps[:, 0:512], lhsT, ws[:, j, 0:512],
                            start=(j == 0), stop=(j == DSUP - 1),
                        )
                        nc.tensor.matmul(
                            ps[:, 512:D_OUT], lhsT, ws[:, j, 512:D_OUT],
                            start=(j == 0), stop=(j == DSUP - 1),
                        )
                    if ds == 0:
                        nc.vector.tensor_copy(cts[nch][:], ps[:])
                    else:
                        nc.vector.tensor_add(cts[nch][:], cts[nch][:], ps[:])
            for nch in range(NCH):
                nc.sync.dma_start(out[nch * P:(nch + 1) * P, :], cts[nch][:])

    nc.compile()
    return nc


def _build_bass_kshard():
    """Design 3 (tensor-parallel): shard the contraction dim d across
    cores (4096 rows each). The W^T shard [4096, 768] (12 MiB) stays
    resident in SBUF; x^T [4096, 8192] streams through once. Each core
    produces a full [8192, 768] partial; the host reduces the 8 partials
    at gather time (the sharding hint's "all-reduce on the [N,768]
    output"). PSUM accumulates the core's entire local contraction."""
    import concourse.mybir as mybir
    import concourse.tile as tile
    from concourse import bacc

    dt_mm = getattr(mybir.dt, MM_DTYPE)
    f32 = mybir.dt.float32
    D_SHARD = D_IN // N_CORES       # 4096 contraction rows per core
    DC = D_SHARD // P               # 32 d-chunks
    NB = N_TOK // N_SUPER           # 16 n-blocks of 512 token rows

    nc = bacc.Bacc(None, target_bir_lowering=False)
    xT = nc.dram_tensor("xT", [D_SHARD, N_TOK], dt_mm, kind="ExternalInput")
    wT = nc.dram_tensor("wT", [D_SHARD, D_OUT], dt_mm, kind="ExternalInput")
    out = nc.dram_tensor("out", [N_TOK, D_OUT], f32, kind="ExternalOutput")

    with tile.TileContext(nc) as tc:
        with (
            tc.tile_pool(name="w", bufs=1) as wpool,
            tc.tile_pool(name="xt", bufs=4) as xpool,
            tc.tile_pool(name="ot", bufs=4) as opool,
            tc.tile_pool(name="psum", bufs=1, space="PSUM") as ppool,
        ):
            ws = wpool.tile([P, DC, D_OUT], dt_mm, name="ws")
            for j in range(DC):
                nc.sync.dma_start(ws[:, j, :], wT[j * P:(j + 1) * P, :])
            for nb in range(NB):
                psums = [
                    ppool.tile([P, D_OUT], f32, name=f"psum{i}")
                    for i in range(N_CH)
                ]
                for dc in range(DC):
                    xt = xpool.tile([P, N_SUPER], dt_mm, name="xt")
                    nc.sync.dma_start(
                        xt[:],
                        xT[dc * P:(dc + 1) * P,
                           nb * N_SUPER:(nb + 1) * N_SUPER],
                    )
                    st = dc == 0
                    sp = dc == DC - 1
                    for nch in range(N_CH):
                        lhsT = xt[:, nch * P:(nch + 1) * P]
                        nc.tensor.matmul(
                            psums[nch][:, 0:512], lhsT, ws[:, dc, 0:512],
                            start=st, stop=sp,
                        )
                        nc.tensor.matmul(
                            psums[nch][:, 512:D_OUT], lhsT,
                            ws[:, dc, 512:D_OUT],
                            start=st, stop=sp,
                        )
                for nch in range(N_CH):
                    ot = opool.tile([P, D_OUT], f32, name="ot")
                    nc.vector.tensor_copy(ot[:], psums[nch][:])
                    base = nb * N_SUPER + nch * P
                    nc.sync.dma_start(out[base:base + P, :], ot[:])

    nc.compile()
    return nc


def _build_bass():
    if DESIGN == "sbuf":
        return _build_bass_sbuf()
    if DESIGN == "kshard":
        return _build_bass_kshard()
    import concourse.mybir as mybir
    import concourse.tile as tile
    from concourse import bacc

    dt_mm = getattr(mybir.dt, MM_DTYPE)
    f32 = mybir.dt.float32

    nc = bacc.Bacc(None, target_bir_lowering=False)
    xT = nc.dram_tensor("xT", [D_IN, N_SHARD], dt_mm, kind="ExternalInput")
    wT = nc.dram_tensor("wT", [D_IN, D_OUT], dt_mm, kind="ExternalInput")
    out = nc.dram_tensor("out", [N_SHARD, D_OUT], f32, kind="ExternalOutput")

    with tile.TileContext(nc) as tc:
        with (
            tc.tile_pool(name="xt", bufs=4) as xpool,
            tc.tile_pool(name="wt", bufs=4) as wpool,
            tc.tile_pool(name="ot", bufs=4) as opool,
            tc.tile_pool(name="psum", bufs=1, space="PSUM") as ppool,
        ):
            for ns in range(N_SUPERS):
                psums = [
                    ppool.tile([P, D_OUT], f32, name=f"psum{i}")
                    for i in range(N_CH)
                ]
                for dc in range(D_CHUNKS):
                    xt = xpool.tile([P, N_SUPER], dt_mm)
                    wt = wpool.tile([P, D_OUT], dt_mm)
                    nc.sync.dma_start(
                        xt[:],
                        xT[dc * P:(dc + 1) * P, ns * N_SUPER:(ns + 1) * N_SUPER],
                    )
                    nc.sync.dma_start(wt[:], wT[dc * P:(dc + 1) * P, :])
                    st = dc == 0
                    sp = dc == D_CHUNKS - 1
                    for nch in range(N_CH):
                        lhsT = xt[:, nch * P:(nch + 1) * P]
                        nc.tensor.matmul(
                            psums[nch][:, 0:512], lhsT, wt[:, 0:512],
                            start=st, stop=sp,
                        )
                        nc.tensor.matmul(
                            psums[nch][:, 512:D_OUT], lhsT, wt[:, 512:D_OUT],
                            start=st, stop=sp,
                        )
                for nch in range(N_CH):
                    ot = opool.tile([P, D_OUT], f32)
                    nc.vector.tensor_copy(ot[:], psums[nch][:])
                    base = ns * N_SUPER + nch * P
                    nc.sync.dma_start(out[base:base + P, :], ot[:])

    nc.compile()
    return nc


def kernel(x: np.ndarray, W: np.ndarray, b_pre: np.ndarray) -> np.ndarray:
    global LAST_RESULTS
    from concourse.bass_utils import run_bass_kernel_spmd

    x = np.asarray(x, dtype=np.float32)
    W = np.asarray(W, dtype=np.float32)
    b_pre = np.asarray(b_pre, dtype=np.float32)

    # Fold the pre-bias on the host (exact no-op for b_pre == 0).
    if b_pre.any():
        x = x - b_pre[None, :]

    wTc = np.ascontiguousarray(W.T)  # [D_IN, D_OUT]
    if DESIGN == "kshard":
        D_SHARD = D_IN // N_CORES
        xTfull = np.ascontiguousarray(x.T)  # [D_IN, N_TOK]
        in_maps = [{
            "xT": xTfull[c * D_SHARD:(c + 1) * D_SHARD],
            "wT": wTc[c * D_SHARD:(c + 1) * D_SHARD],
        } for c in range(N_CORES)]
    else:
        in_maps = [{
            "xT": np.ascontiguousarray(x[c * N_SHARD:(c + 1) * N_SHARD].T),
            "wT": wTc,
        } for c in range(N_CORES)]

    nc = _build_bass()
    LAST_RESULTS = run_bass_kernel_spmd(
        nc, in_maps, core_ids=list(range(N_CORES)),
        tmpdir=os.environ.get("KERNEL_TRACE_DIR") or None,
    )
    if DESIGN == "kshard":
        # Tensor-parallel: reduce the per-core partials (host all-reduce).
        acc = np.zeros((N_TOK, D_OUT), dtype=np.float64)
        for c in range(N_CORES):
            acc += LAST_RESULTS.results[c]["out"]
        return acc.astype(np.float32)
    return np.concatenate(
        [LAST_RESULTS.results[c]["out"] for c in range(N_CORES)], axis=0
    )
